# revision 2
# baseline (speedup 1.0000x reference)
"""Trainium2 Bass kernel: per-row InstanceNorm + Linear(512->512) + ReLU.

Computes, for x [N, 512], W [512, 512], b [512]:
    xn = (x - mean_row) * rsqrt(var_row + 1e-5)      (biased var, per row)
    y  = relu(xn @ W.T + b)

v2 strategy (vs v1 baseline at ~354 us):
  - bf16 I/O both directions (host casts): halves HBM traffic.
  - Output computed and stored transposed (y.T, [512, rows]): the Linear
    weight W.T becomes the stationary matmul operand and the bias becomes a
    per-partition ACT scalar, so the bias matmul disappears from PE.
    The host un-transposes at the end.
  - The xn transpose (contraction dim onto partitions) is split: 2 of the 4
    128-col chunks go through the DMA XBAR transpose (SBUF->SBUF), 2 through
    PE transposes, balancing PE cycles against DMA-fabric bytes.
  - Normalize runs on the (otherwise idle) Pool/GpSimd engine; stats (DVE
    bn_stats/bn_aggr), sqrt (ACT, batched), reciprocal (DVE, batched).

Per core: 25600 rows in 25 groups of 1024 rows (8 row-tiles of 128).
"""

import os
import sys

import numpy as np

sys.path.insert(0, "/opt/trn_rl_repo")

import ml_dtypes  # noqa: E402

import concourse.bacc as bacc  # noqa: E402
import concourse.bass as bass  # noqa: E402
import concourse.tile as tile  # noqa: E402
from concourse import mybir  # noqa: E402
from concourse.bass_utils import run_bass_kernel_spmd  # noqa: E402

N_CORES = 8
N_FULL = 200000
N_IN = 512
N_OUT = 512
P = 128
KC = N_IN // P  # 4 contraction chunks
OC = N_OUT // P  # 4 output chunks
TJ = 8  # row-tiles per group
RG = P * TJ  # rows per group = 1024
NQ = 2  # matmul row-streams (supers) per group, 512 rows each
GROUPS = 25
ROWS_PER_CORE = GROUPS * RG  # 25600
N_PAD = ROWS_PER_CORE * N_CORES  # 204800

PE_CHUNKS = (0, 1)  # xn chunks transposed on PE
XBAR_CHUNKS = (2, 3)  # xn chunks transposed via DMA XBAR

EPS = 1e-5

F32 = mybir.dt.float32
BF16 = mybir.dt.bfloat16

LAST_RUN = None  # BassKernelResults of the most recent run (for test harness)


def build_bass() -> bass.Bass:
    nc = bacc.Bacc()
    x_d = nc.declare_dram_parameter("x", [ROWS_PER_CORE, N_IN], BF16, isOutput=False)
    wt_d = nc.declare_dram_parameter("wt", [P, KC * OC * P], BF16, isOutput=False)
    b_d = nc.declare_dram_parameter("bvec", [P, OC], F32, isOutput=False)
    ident_d = nc.declare_dram_parameter("ident", [P, P], BF16, isOutput=False)
    y_d = nc.declare_dram_parameter("y", [N_OUT, ROWS_PER_CORE], BF16, isOutput=True)

    npe = len(PE_CHUNKS)

    with tile.TileContext(nc) as tc:
        with (
            tc.tile_pool(name="singles", bufs=1) as singles,
            tc.tile_pool(name="xin", bufs=3) as xin_pool,
            tc.tile_pool(name="stats", bufs=3) as stats_pool,
            tc.tile_pool(name="xn", bufs=3) as xn_pool,
            tc.tile_pool(name="xnt", bufs=2) as xnt_pool,
            tc.tile_pool(name="yout", bufs=2) as y_pool,
            tc.tile_pool(name="pst", bufs=3, space="PSUM") as pst_pool,
            tc.tile_pool(name="psy", bufs=2, space="PSUM") as psy_pool,
        ):
            # --- constants (loaded once) ---
            # wt_sb[p, kc, oc, o] = W[oc*128+o, kc*128+p]
            wt_sb = singles.tile([P, KC, OC, P], BF16)
            nc.sync.dma_start(out=wt_sb, in_=wt_d[:, :])
            ident_sb = singles.tile([P, P], BF16)
            nc.sync.dma_start(out=ident_sb, in_=ident_d[:, :])
            bt_sb = singles.tile([P, OC], F32)  # bt[p, oc] = b[oc*128+p]
            nc.sync.dma_start(out=bt_sb, in_=b_d[:, :])
            eps_sb = singles.tile([P, 1], F32)
            nc.vector.memset(eps_sb, EPS)

            # group g, partition p, slot j  <->  row g*1024 + p*8 + j
            x_b = x_d[:, :].rearrange("(g p j) i -> g p j i", p=P, j=TJ)
            # store: yT[o, g*1024 + q*512 + j'*128 + r] = y.T as computed
            y_b = y_d[:, :].rearrange("(oc p) (g r) -> g p oc r", p=P, r=RG)

            for g in range(GROUPS):
                xg = xin_pool.tile([P, TJ, N_IN], BF16)
                nc.sync.dma_start(out=xg, in_=x_b[g])

                mvs = stats_pool.tile([P, TJ, 2], F32)
                for j in range(TJ):
                    st6 = stats_pool.tile([P, 6], F32)
                    nc.vector.bn_stats(out=st6, in_=xg[:, j, :])
                    nc.vector.bn_aggr(out=mvs[:, j, :], in_=st6)
                sd = stats_pool.tile([P, TJ], F32)
                nc.scalar.activation(
                    out=sd, in_=mvs[:, :, 1],
                    func=mybir.ActivationFunctionType.Sqrt,
                    bias=eps_sb[:, :], scale=1.0,
                )
                rstd = stats_pool.tile([P, TJ], F32)
                nc.vector.reciprocal(out=rstd, in_=sd)

                xn = xn_pool.tile([P, TJ, N_IN], BF16)
                xnt = xnt_pool.tile([P, KC, TJ, P], BF16)
                for j in range(TJ):
                    nc.gpsimd.tensor_scalar(
                        out=xn[:, j, :], in0=xg[:, j, :],
                        scalar1=mvs[:, j, 0:1], scalar2=rstd[:, j:j + 1],
                        op0=mybir.AluOpType.subtract, op1=mybir.AluOpType.mult,
                    )
                    pst = pst_pool.tile([P, npe, P], BF16)
                    for ci, c in enumerate(PE_CHUNKS):
                        nc.tensor.transpose(
                            pst[:, ci, :], xn[:, j, c * P:(c + 1) * P], ident_sb[:, :]
                        )
                    nc.vector.tensor_copy(
                        out=xnt[:, PE_CHUNKS[0]:PE_CHUNKS[-1] + 1, j, :], in_=pst
                    )
                    for c in XBAR_CHUNKS:
                        nc.sync.dma_start(
                            out=xnt[:, c, j, :], in_=xn[:, j, c * P:(c + 1) * P],
                            transpose=True,
                        )

                yt = y_pool.tile([P, OC, RG], BF16)
                for q in range(NQ):
                    for oph in range(OC // 2):
                        ps = psy_pool.tile([P, 2, N_OUT], F32)
                        for ol in range(2):
                            oc = oph * 2 + ol
                            for kc in range(KC):
                                nc.tensor.matmul(
                                    ps[:, ol, :],
                                    wt_sb[:, kc, oc, :],
                                    xnt[:, kc, 4 * q:4 * q + 4, :],
                                    start=(kc == 0),
                                    stop=(kc == KC - 1),
                                )
                            nc.scalar.activation(
                                out=yt[:, oc, q * 512:(q + 1) * 512],
                                in_=ps[:, ol, :],
                                func=mybir.ActivationFunctionType.Relu,
                                bias=bt_sb[:, oc:oc + 1], scale=1.0,
                            )
                nc.sync.dma_start(out=y_b[g], in_=yt)
    nc.compile()
    return nc


_BASS_CACHE: dict[str, bass.Bass] = {}


def _get_bass() -> bass.Bass:
    if "k" not in _BASS_CACHE:
        _BASS_CACHE["k"] = build_bass()
    return _BASS_CACHE["k"]


def kernel(x: np.ndarray, W: np.ndarray, b: np.ndarray) -> np.ndarray:
    global LAST_RUN
    x = np.asarray(x, dtype=np.float32)
    W = np.asarray(W, dtype=np.float32)
    b = np.asarray(b, dtype=np.float32)
    n = x.shape[0]

    nc = _get_bass()

    x_pad = np.zeros((N_PAD, N_IN), dtype=ml_dtypes.bfloat16)
    x_pad[:n] = x.astype(ml_dtypes.bfloat16)
    # wt[p, kc, oc, o] = W[oc*128+o, kc*128+p]
    wt = np.ascontiguousarray(
        W.reshape(OC, P, KC, P).transpose(3, 2, 0, 1).reshape(P, KC * OC * P)
    ).astype(ml_dtypes.bfloat16)
    bt = np.ascontiguousarray(b.reshape(OC, P).T)  # [128, OC] f32
    ident = np.eye(P, dtype=ml_dtypes.bfloat16)

    in_maps = [
        {
            "x": np.ascontiguousarray(x_pad[c * ROWS_PER_CORE:(c + 1) * ROWS_PER_CORE]),
            "wt": wt,
            "bvec": bt,
            "ident": ident,
        }
        for c in range(N_CORES)
    ]
    trace = bool(os.environ.get("BASS_TRACE"))
    res = run_bass_kernel_spmd(nc, in_maps, list(range(N_CORES)), trace=trace)
    LAST_RUN = res

    out = np.empty((n, N_OUT), dtype=np.float32)
    done = 0
    for c in range(N_CORES):
        if done >= n:
            break
        yt = np.asarray(res.results[c]["y"])  # [512, ROWS_PER_CORE] bf16
        # column index = g*1024 + q*512 + j'*128 + r  <->  row g*1024 + r*8 + q*4 + j'
        y_core = (
            yt.reshape(N_OUT, GROUPS, NQ, 4, P)
            .transpose(1, 4, 2, 3, 0)
            .reshape(ROWS_PER_CORE, N_OUT)
            .astype(np.float32)
        )
        take = min(ROWS_PER_CORE, n - done)
        out[done:done + take] = y_core[:take]
        done += take
    return out


# revision 5
# speedup vs baseline: 2.6881x; 2.6881x over previous
"""Trainium2 Bass kernel: per-row InstanceNorm + Linear(512->512) + ReLU.

Computes, for x [N, 512], W [512, 512], b [512]:
    xn = (x - mean_row) * rsqrt(var_row + 1e-5)      (biased var, per row)
    y  = relu(xn @ W.T + b)

Strategy (v3.1, baseline v1 ~354us):
  - bf16 I/O both directions (host casts): halves HBM traffic.
  - y computed transposed (y.T [512, rows]): W.T chunks are the stationary
    matmul operand and the bias rides the ACT evacuation as a per-partition
    scalar (no bias matmul). Host un-transposes the output.
  - Per-oc PSUM tile [128, 2, 512] holds both 512-row streams so one wide
    ACT Relu+bias evacuation covers [128, 1024]; matmul order (oc, kc, q)
    reuses each stationary W tile for 2 consecutive matmuls.
  - xn transpose: chunks 0-1 via PE transposes (pipelined, ~55ns) + a
    DVE/ACT PSUM->SBUF copy; chunks 2-3 via one XBAR DMA transpose per tile
    (runs on the DMA engines concurrently with PE).
  - Stats: DVE bn_stats/bn_aggr per tile; sqrt(ACT)/reciprocal(DVE) batched
    per group. Normalize via tensor_scalar (2x mode), split DVE/ACT.

Per core: 25600 rows in 25 groups of 1024 rows (8 row-tiles of 128).
Row mapping: row = g*1024 + p*8 + j (partition p, slot j).
"""

import os
import sys

import numpy as np

sys.path.insert(0, "/opt/trn_rl_repo")

import ml_dtypes  # noqa: E402

import concourse.bacc as bacc  # noqa: E402
import concourse.bass as bass  # noqa: E402
import concourse.tile as tile  # noqa: E402
from concourse import mybir  # noqa: E402
from concourse.bass_utils import run_bass_kernel_spmd  # noqa: E402

N_CORES = 8
N_FULL = 200000
N_IN = 512
N_OUT = 512
P = 128
KC = N_IN // P  # 4 contraction chunks
OC = N_OUT // P  # 4 output chunks
TJ = 8  # row-tiles per group
RG = P * TJ  # rows per group = 1024
NQ = 2  # matmul row-streams (supers) per group, 512 rows each
GROUPS = 25
ROWS_PER_CORE = GROUPS * RG  # 25600
N_PAD = ROWS_PER_CORE * N_CORES  # 204800

NPE = 2  # chunks transposed on PE (0..NPE-1); rest via XBAR DMA
NORM_ON_ACT = (1, 4, 6)
COPY_ON_ACT = (0, 3, 5, 7)

EPS = 1e-5

F32 = mybir.dt.float32
BF16 = mybir.dt.bfloat16

LAST_RUN = None  # BassKernelResults of the most recent run (for test harness)


def build_bass() -> bass.Bass:
    nc = bacc.Bacc()
    x_d = nc.declare_dram_parameter("x", [ROWS_PER_CORE, N_IN], BF16, isOutput=False)
    wt_d = nc.declare_dram_parameter("wt", [P, KC * OC * P], BF16, isOutput=False)
    b_d = nc.declare_dram_parameter("bvec", [P, OC], F32, isOutput=False)
    ident_d = nc.declare_dram_parameter("ident", [P, P], BF16, isOutput=False)
    y_d = nc.declare_dram_parameter("y", [N_OUT, ROWS_PER_CORE], BF16, isOutput=True)

    with tile.TileContext(nc) as tc:
        with (
            tc.tile_pool(name="singles", bufs=1) as singles,
            tc.tile_pool(name="xin", bufs=3) as xin_pool,
            tc.tile_pool(name="stats", bufs=3) as stats_pool,
            tc.tile_pool(name="xn", bufs=3) as xn_pool,
            tc.tile_pool(name="xnt", bufs=2) as xnt_pool,
            tc.tile_pool(name="yout", bufs=2) as y_pool,
            tc.tile_pool(name="pst", bufs=3, space="PSUM") as pst_pool,
            tc.tile_pool(name="psy", bufs=2, space="PSUM") as psy_pool,
        ):
            # --- constants (loaded once) ---
            # wt_sb[p, kc, oc, o] = W[oc*128+o, kc*128+p]
            wt_sb = singles.tile([P, KC, OC, P], BF16)
            nc.sync.dma_start(out=wt_sb, in_=wt_d[:, :])
            ident_sb = singles.tile([P, P], BF16)
            nc.sync.dma_start(out=ident_sb, in_=ident_d[:, :])
            bt_sb = singles.tile([P, OC], F32)  # bt[p, oc] = b[oc*128+p]
            nc.sync.dma_start(out=bt_sb, in_=b_d[:, :])
            eps_sb = singles.tile([P, 1], F32)
            nc.vector.memset(eps_sb, EPS)

            # group g, partition p, slot j  <->  row g*1024 + p*8 + j
            x_b = x_d[:, :].rearrange("(g p j) i -> g p j i", p=P, j=TJ)
            # store: yT[oc*128+p, g*1024 + q*512 + j'*128 + r]
            y_b = y_d[:, :].rearrange("(oc p) (g r) -> g p oc r", p=P, r=RG)

            for g in range(GROUPS):
                xg = xin_pool.tile([P, TJ, N_IN], BF16)
                nc.sync.dma_start(out=xg, in_=x_b[g])

                # --- stats ---
                mvs = stats_pool.tile([P, TJ, 2], F32)
                for j in range(TJ):
                    st6 = stats_pool.tile([P, 6], F32)
                    nc.vector.bn_stats(out=st6, in_=xg[:, j, :])
                    nc.vector.bn_aggr(out=mvs[:, j, :], in_=st6)
                sd = stats_pool.tile([P, TJ], F32)
                nc.scalar.activation(
                    out=sd, in_=mvs[:, :, 1],
                    func=mybir.ActivationFunctionType.Sqrt,
                    bias=eps_sb[:, :], scale=1.0,
                )
                rstd = stats_pool.tile([P, TJ], F32)
                nc.vector.reciprocal(out=rstd, in_=sd)
                # negmrs = -mean*rstd (bias for ACT-side normalize)
                negmrs = stats_pool.tile([P, TJ], F32)
                nc.vector.scalar_tensor_tensor(
                    out=negmrs, in0=mvs[:, :, 0], scalar=-1.0, in1=rstd,
                    op0=mybir.AluOpType.mult, op1=mybir.AluOpType.mult,
                )

                # --- normalize + transpose per tile ---
                xn = xn_pool.tile([P, TJ, N_IN], BF16)
                xnt = xnt_pool.tile([P, TJ, KC, P], BF16)
                for j in range(TJ):
                    if j in NORM_ON_ACT:
                        nc.scalar.activation(
                            out=xn[:, j, :], in_=xg[:, j, :],
                            func=mybir.ActivationFunctionType.Identity,
                            bias=negmrs[:, j:j + 1], scale=rstd[:, j:j + 1],
                        )
                    else:
                        nc.vector.tensor_scalar(
                            out=xn[:, j, :], in0=xg[:, j, :],
                            scalar1=mvs[:, j, 0:1], scalar2=rstd[:, j:j + 1],
                            op0=mybir.AluOpType.subtract, op1=mybir.AluOpType.mult,
                        )
                    # chunks 0..NPE-1 on PE
                    pst = pst_pool.tile([P, NPE, P], BF16)
                    for c in range(NPE):
                        nc.tensor.transpose(
                            pst[:, c, :], xn[:, j, c * P:(c + 1) * P], ident_sb[:, :]
                        )
                    if j in COPY_ON_ACT:
                        nc.scalar.copy(out=xnt[:, j, 0:NPE, :], in_=pst)
                    else:
                        nc.vector.tensor_copy(out=xnt[:, j, 0:NPE, :], in_=pst)
                    # chunks NPE..KC-1 via XBAR (one DMA transpose per tile)
                    nc.sync.dma_start(
                        out=xnt[:, j, NPE:KC, :],
                        in_=xn[:, j, NPE * P:KC * P],
                        transpose=True,
                    )

                # --- matmuls (W stationary, reused across q) + wide evac ---
                yt = y_pool.tile([P, OC, RG], BF16)
                for oc in range(OC):
                    ps = psy_pool.tile([P, NQ, N_OUT], F32)
                    for kc in range(KC):
                        for q in range(NQ):
                            nc.tensor.matmul(
                                ps[:, q, :],
                                wt_sb[:, kc, oc, :],
                                xnt[:, 4 * q:4 * q + 4, kc, :],
                                start=(kc == 0),
                                stop=(kc == KC - 1),
                            )
                    nc.scalar.activation(
                        out=yt[:, oc, :],
                        in_=ps[:, :, :],
                        func=mybir.ActivationFunctionType.Relu,
                        bias=bt_sb[:, oc:oc + 1], scale=1.0,
                    )
                nc.sync.dma_start(out=y_b[g], in_=yt)
    nc.compile()
    return nc


_BASS_CACHE: dict[str, bass.Bass] = {}


def _get_bass() -> bass.Bass:
    if "k" not in _BASS_CACHE:
        _BASS_CACHE["k"] = build_bass()
    return _BASS_CACHE["k"]


def kernel(x: np.ndarray, W: np.ndarray, b: np.ndarray) -> np.ndarray:
    global LAST_RUN
    x = np.asarray(x, dtype=np.float32)
    W = np.asarray(W, dtype=np.float32)
    b = np.asarray(b, dtype=np.float32)
    n = x.shape[0]

    nc = _get_bass()

    x_pad = np.zeros((N_PAD, N_IN), dtype=ml_dtypes.bfloat16)
    x_pad[:n] = x.astype(ml_dtypes.bfloat16)
    # wt[p, kc, oc, o] = W[oc*128+o, kc*128+p]
    wt = np.ascontiguousarray(
        W.reshape(OC, P, KC, P).transpose(3, 2, 0, 1).reshape(P, KC * OC * P)
    ).astype(ml_dtypes.bfloat16)
    bt = np.ascontiguousarray(b.reshape(OC, P).T)  # [128, OC] f32
    ident = np.eye(P, dtype=ml_dtypes.bfloat16)

    in_maps = [
        {
            "x": np.ascontiguousarray(x_pad[c * ROWS_PER_CORE:(c + 1) * ROWS_PER_CORE]),
            "wt": wt,
            "bvec": bt,
            "ident": ident,
        }
        for c in range(N_CORES)
    ]
    trace = bool(os.environ.get("BASS_TRACE"))
    res = run_bass_kernel_spmd(nc, in_maps, list(range(N_CORES)), trace=trace)
    LAST_RUN = res

    out = np.empty((n, N_OUT), dtype=np.float32)
    done = 0
    for c in range(N_CORES):
        if done >= n:
            break
        yt = np.asarray(res.results[c]["y"])  # [512, ROWS_PER_CORE] bf16
        # col = g*1024 + q*512 + j'*128 + r  <->  row = g*1024 + r*8 + q*4 + j'
        y_core = (
            yt.reshape(N_OUT, GROUPS, NQ, 4, P)
            .transpose(1, 4, 2, 3, 0)
            .reshape(ROWS_PER_CORE, N_OUT)
            .astype(np.float32)
        )
        take = min(ROWS_PER_CORE, n - done)
        out[done:done + take] = y_core[:take]
        done += take
    return out


# revision 7
# speedup vs baseline: 5.7473x; 2.1381x over previous
"""Trainium2 Bass kernel: per-row InstanceNorm + Linear(512->512) + ReLU.

Computes, for x [N, 512], W [512, 512], b [512]:
    xn = (x - mean_row) * rsqrt(var_row + 1e-5)      (biased var, per row)
    y  = relu(xn @ W.T + b)

Strategy (v3.1, baseline v1 ~354us):
  - bf16 I/O both directions (host casts): halves HBM traffic.
  - y computed transposed (y.T [512, rows]): W.T chunks are the stationary
    matmul operand and the bias rides the ACT evacuation as a per-partition
    scalar (no bias matmul). Host un-transposes the output.
  - Per-oc PSUM tile [128, 2, 512] holds both 512-row streams so one wide
    ACT Relu+bias evacuation covers [128, 1024]; matmul order (oc, kc, q)
    reuses each stationary W tile for 2 consecutive matmuls.
  - xn transpose: chunks 0-1 via PE transposes (pipelined, ~55ns) + a
    DVE/ACT PSUM->SBUF copy; chunks 2-3 via one XBAR DMA transpose per tile
    (runs on the DMA engines concurrently with PE).
  - Stats: DVE bn_stats/bn_aggr per tile; sqrt(ACT)/reciprocal(DVE) batched
    per group. Normalize via tensor_scalar (2x mode), split DVE/ACT.

Per core: 25600 rows in 25 groups of 1024 rows (8 row-tiles of 128).
Row mapping: row = g*1024 + p*8 + j (partition p, slot j).
"""

import os
import sys

import numpy as np

sys.path.insert(0, "/opt/trn_rl_repo")

import ml_dtypes  # noqa: E402

import concourse.bacc as bacc  # noqa: E402
import concourse.bass as bass  # noqa: E402
import concourse.tile as tile  # noqa: E402
from concourse import mybir  # noqa: E402
from concourse.bass_utils import run_bass_kernel_spmd  # noqa: E402

N_CORES = 8
N_FULL = 200000
N_IN = 512
N_OUT = 512
P = 128
KC = N_IN // P  # 4 contraction chunks
OC = N_OUT // P  # 4 output chunks
TJ = 8  # row-tiles per group
RG = P * TJ  # rows per group = 1024
NQ = 2  # matmul row-streams (supers) per group, 512 rows each
GROUPS = 25
ROWS_PER_CORE = GROUPS * RG  # 25600
N_PAD = ROWS_PER_CORE * N_CORES  # 204800

NORM_ON_ACT = (1, 3, 5, 6)  # tiles whose normalize runs on ACT
COPY_ON_ACT = (1, 3)  # tile-pairs whose psum->sbuf copy runs on ACT

EPS = 1e-5

F32 = mybir.dt.float32
BF16 = mybir.dt.bfloat16

LAST_RUN = None  # BassKernelResults of the most recent run (for test harness)


def build_bass() -> bass.Bass:
    nc = bacc.Bacc()
    x_d = nc.declare_dram_parameter("x", [ROWS_PER_CORE, N_IN], BF16, isOutput=False)
    wt_d = nc.declare_dram_parameter("wt", [P, KC * OC * P], BF16, isOutput=False)
    b_d = nc.declare_dram_parameter("bvec", [P, OC], F32, isOutput=False)
    ident_d = nc.declare_dram_parameter("ident", [P, P], BF16, isOutput=False)
    y_d = nc.declare_dram_parameter("y", [N_OUT, ROWS_PER_CORE], BF16, isOutput=True)

    with tile.TileContext(nc) as tc:
        with (
            tc.tile_pool(name="singles", bufs=1) as singles,
            tc.tile_pool(name="xin", bufs=3) as xin_pool,
            tc.tile_pool(name="stats", bufs=3) as stats_pool,
            tc.tile_pool(name="xn", bufs=3) as xn_pool,
            tc.tile_pool(name="xnt", bufs=2) as xnt_pool,
            tc.tile_pool(name="yout", bufs=2) as y_pool,
            tc.tile_pool(name="pst", bufs=3, space="PSUM") as pst_pool,
            tc.tile_pool(name="psy", bufs=2, space="PSUM") as psy_pool,
        ):
            # --- constants (loaded once) ---
            # wt_sb[p, kc, oc, o] = W[oc*128+o, kc*128+p]
            wt_sb = singles.tile([P, KC, OC, P], BF16)
            nc.sync.dma_start(out=wt_sb, in_=wt_d[:, :])
            ident_sb = singles.tile([P, P], BF16)
            nc.sync.dma_start(out=ident_sb, in_=ident_d[:, :])
            bt_sb = singles.tile([P, OC], F32)  # bt[p, oc] = b[oc*128+p]
            nc.sync.dma_start(out=bt_sb, in_=b_d[:, :])
            eps_sb = singles.tile([P, 1], F32)
            nc.vector.memset(eps_sb, EPS)

            # group g, partition p, slot j  <->  row g*1024 + p*8 + j
            x_b = x_d[:, :].rearrange("(g p j) i -> g p j i", p=P, j=TJ)
            # store: yT[oc*128+p, g*1024 + q*512 + j'*128 + r]
            y_b = y_d[:, :].rearrange("(oc p) (g r) -> g p oc r", p=P, r=RG)

            for g in range(GROUPS):
                xg = xin_pool.tile([P, TJ, N_IN], BF16)
                nc.sync.dma_start(out=xg, in_=x_b[g])

                # --- stats ---
                mvs = stats_pool.tile([P, TJ, 2], F32)
                for j in range(TJ):
                    st6 = stats_pool.tile([P, 6], F32)
                    nc.vector.bn_stats(out=st6, in_=xg[:, j, :])
                    nc.vector.bn_aggr(out=mvs[:, j, :], in_=st6)
                sd = stats_pool.tile([P, TJ], F32)
                nc.scalar.activation(
                    out=sd, in_=mvs[:, :, 1],
                    func=mybir.ActivationFunctionType.Sqrt,
                    bias=eps_sb[:, :], scale=1.0,
                )
                rstd = stats_pool.tile([P, TJ], F32)
                nc.vector.reciprocal(out=rstd, in_=sd)
                # negmrs = -mean*rstd (bias for ACT-side normalize)
                negmrs = stats_pool.tile([P, TJ], F32)
                nc.vector.scalar_tensor_tensor(
                    out=negmrs, in0=mvs[:, :, 0], scalar=-1.0, in1=rstd,
                    op0=mybir.AluOpType.mult, op1=mybir.AluOpType.mult,
                )

                # --- normalize + transpose per tile; copies per tile-pair ---
                xn = xn_pool.tile([P, TJ, N_IN], BF16)
                xnt = xnt_pool.tile([P, TJ, KC, P], BF16)
                for jp in range(TJ // 2):
                    pst = pst_pool.tile([P, 2, KC, P], BF16)
                    for jl in range(2):
                        j = 2 * jp + jl
                        if j in NORM_ON_ACT:
                            nc.scalar.activation(
                                out=xn[:, j, :], in_=xg[:, j, :],
                                func=mybir.ActivationFunctionType.Identity,
                                bias=negmrs[:, j:j + 1], scale=rstd[:, j:j + 1],
                            )
                        else:
                            nc.vector.tensor_scalar(
                                out=xn[:, j, :], in0=xg[:, j, :],
                                scalar1=mvs[:, j, 0:1], scalar2=rstd[:, j:j + 1],
                                op0=mybir.AluOpType.subtract,
                                op1=mybir.AluOpType.mult,
                            )
                        for c in range(KC):
                            nc.tensor.transpose(
                                pst[:, jl, c, :], xn[:, j, c * P:(c + 1) * P],
                                ident_sb[:, :],
                            )
                    if jp in COPY_ON_ACT:
                        nc.scalar.copy(out=xnt[:, 2 * jp:2 * jp + 2, :, :], in_=pst)
                    else:
                        nc.vector.tensor_copy(
                            out=xnt[:, 2 * jp:2 * jp + 2, :, :], in_=pst
                        )

                # --- matmuls (W stationary, reused across q) + wide evac ---
                yt = y_pool.tile([P, OC, RG], BF16)
                for oc in range(OC):
                    ps = psy_pool.tile([P, NQ, N_OUT], F32)
                    for kc in range(KC):
                        for q in range(NQ):
                            nc.tensor.matmul(
                                ps[:, q, :],
                                wt_sb[:, kc, oc, :],
                                xnt[:, 4 * q:4 * q + 4, kc, :],
                                start=(kc == 0),
                                stop=(kc == KC - 1),
                            )
                    nc.scalar.activation(
                        out=yt[:, oc, :],
                        in_=ps[:, :, :],
                        func=mybir.ActivationFunctionType.Relu,
                        bias=bt_sb[:, oc:oc + 1], scale=1.0,
                    )
                nc.sync.dma_start(out=y_b[g], in_=yt)
    nc.compile()
    return nc


_BASS_CACHE: dict[str, bass.Bass] = {}


def _get_bass() -> bass.Bass:
    if "k" not in _BASS_CACHE:
        _BASS_CACHE["k"] = build_bass()
    return _BASS_CACHE["k"]


def kernel(x: np.ndarray, W: np.ndarray, b: np.ndarray) -> np.ndarray:
    global LAST_RUN
    x = np.asarray(x, dtype=np.float32)
    W = np.asarray(W, dtype=np.float32)
    b = np.asarray(b, dtype=np.float32)
    n = x.shape[0]

    nc = _get_bass()

    x_pad = np.zeros((N_PAD, N_IN), dtype=ml_dtypes.bfloat16)
    x_pad[:n] = x.astype(ml_dtypes.bfloat16)
    # wt[p, kc, oc, o] = W[oc*128+o, kc*128+p]
    wt = np.ascontiguousarray(
        W.reshape(OC, P, KC, P).transpose(3, 2, 0, 1).reshape(P, KC * OC * P)
    ).astype(ml_dtypes.bfloat16)
    bt = np.ascontiguousarray(b.reshape(OC, P).T)  # [128, OC] f32
    ident = np.eye(P, dtype=ml_dtypes.bfloat16)

    in_maps = [
        {
            "x": np.ascontiguousarray(x_pad[c * ROWS_PER_CORE:(c + 1) * ROWS_PER_CORE]),
            "wt": wt,
            "bvec": bt,
            "ident": ident,
        }
        for c in range(N_CORES)
    ]
    trace = bool(os.environ.get("BASS_TRACE"))
    res = run_bass_kernel_spmd(nc, in_maps, list(range(N_CORES)), trace=trace)
    LAST_RUN = res

    out = np.empty((n, N_OUT), dtype=np.float32)
    done = 0
    for c in range(N_CORES):
        if done >= n:
            break
        yt = np.asarray(res.results[c]["y"])  # [512, ROWS_PER_CORE] bf16
        # col = g*1024 + q*512 + j'*128 + r  <->  row = g*1024 + r*8 + q*4 + j'
        y_core = (
            yt.reshape(N_OUT, GROUPS, NQ, 4, P)
            .transpose(1, 4, 2, 3, 0)
            .reshape(ROWS_PER_CORE, N_OUT)
            .astype(np.float32)
        )
        take = min(ROWS_PER_CORE, n - done)
        out[done:done + take] = y_core[:take]
        done += take
    return out
